# revision 1
# baseline (speedup 1.0000x reference)
"""Distributed TransformerConv GNN (2 layers + FC + log_softmax) on 8 trn2 cores.

Sharding: nodes partitioned by destination across 8 cores (6250 own nodes each,
padded to 6272 = 49x128). Edges sharded by dst, sorted by dst on host. Per layer:
each core computes q/k/v/s projections for its own nodes, AllGathers the k|v
table, then processes its edges in 128-edge chunks: indirect-DMA gather of kv
rows by src, PE-transpose k, PE matmul scores against blockwise q^T, exp on ACT,
one-hot dst mask (iota compare), masked-exp weights, and PE matmul accumulation
of both the weighted-v aggregate and the softmax denominator in PSUM.
No segment-max is needed: scores are O(1) here, so softmax without max
subtraction is mathematically identical and fp32-safe.
"""

import sys

sys.path.insert(0, "/opt/trn_rl_repo")

import numpy as np

from concourse import bacc, bass, mybir, tile
from concourse import bass_utils

N = 50000
E = 600000
F = 128
C = 10
L = 2
M = 8  # cores
NO = N // M  # 6250 own real nodes
P = 128
NB = (NO + P - 1) // P  # 49 blocks
NOP = NB * P  # 6272 padded own nodes
NPAD = M * NOP  # 50176
SCALE = 1.0 / np.sqrt(128.0)

F32 = mybir.dt.float32
I32 = mybir.dt.int32

_cache = {}


def _host_prep(edge_index):
    """Sort/bucket edges by dst; build per-core [128, NCH] src-index and
    dst-local tables (column j = chunk j, chunk j = block*(CMAX)+c)."""
    src = np.asarray(edge_index[0], dtype=np.int64)
    dst = np.asarray(edge_index[1], dtype=np.int64)
    src_pad = (src // NO) * NOP + (src % NO)  # remap to padded node ids

    core_of = dst // NO
    srctabs, dsttabs = [], []
    percore = []
    cmax = 1
    for c in range(M):
        sel = core_of == c
        d_loc = (dst[sel] - c * NO).astype(np.int64)
        s_glob = src_pad[sel]
        order = np.argsort(d_loc, kind="stable")
        d_loc = d_loc[order]
        s_glob = s_glob[order]
        cnt = np.bincount(d_loc // P, minlength=NB)
        cmax = max(cmax, int(np.max((cnt + P - 1) // P)))
        percore.append((d_loc, s_glob, cnt))
    for c in range(M):
        d_loc, s_glob, cnt = percore[c]
        nch = NB * cmax
        srctab = np.zeros((nch, P), dtype=np.int32)
        dsttab = np.full((nch, P), -1.0, dtype=np.float32)
        starts = np.concatenate([[0], np.cumsum(cnt)])
        for b in range(NB):
            e0, e1 = starts[b], starts[b + 1]
            n = e1 - e0
            if n == 0:
                continue
            flat_s = np.zeros(cmax * P, dtype=np.int32)
            flat_d = np.full(cmax * P, -1.0, dtype=np.float32)
            flat_s[:n] = s_glob[e0:e1]
            flat_d[:n] = (d_loc[e0:e1] % P).astype(np.float32)
            srctab[b * cmax : (b + 1) * cmax] = flat_s.reshape(cmax, P)
            dsttab[b * cmax : (b + 1) * cmax] = flat_d.reshape(cmax, P)
        # transpose so column j = chunk j, partition p = edge p of chunk
        srctabs.append(np.ascontiguousarray(srctab.T))
        dsttabs.append(np.ascontiguousarray(dsttab.T))
    return cmax, srctabs, dsttabs


def _build(cmax):
    nch = NB * cmax
    nc = bacc.Bacc("TRN2", target_bir_lowering=False, debug=False, num_devices=M)

    def din(name, shape, dt=F32):
        return nc.dram_tensor(name, list(shape), dt, kind="ExternalInput").ap()

    xT = din("xT", [P, NOP])
    srctab = din("srctab", [P, nch], I32)
    dsttab = din("dsttab", [P, nch])
    wqt = din("wqt", [L, F, F])
    wkt = din("wkt", [L, F, F])
    wvt = din("wvt", [L, F, F])
    wst = din("wst", [L, F, F])
    bqr = din("bqr", [L, 1, F])
    bkr = din("bkr", [L, 1, F])
    bvr = din("bvr", [L, 1, F])
    bsr = din("bsr", [L, 1, F])
    fcwt = din("fcwt", [F, C])
    fcbr = din("fcbr", [1, C])
    iota_in = din("iota", [P, P])
    ident_in = din("ident", [P, P])
    out = nc.dram_tensor("out", [NOP, C], F32, kind="ExternalOutput").ap()

    kv_own = nc.dram_tensor("kv_own", [NOP, 2 * F + 1], F32)
    kv_all = nc.dram_tensor("kv_all", [NPAD, 2 * F + 1], F32, addr_space="Shared")

    groups = [list(range(M))]

    with tile.TileContext(nc) as tc:
        with (
            tc.tile_pool(name="const", bufs=1) as cpool,
            tc.tile_pool(name="big", bufs=1) as bigp,
            tc.tile_pool(name="work", bufs=4) as work,
            tc.tile_pool(name="kvpool", bufs=6) as kvp,
            tc.tile_pool(name="ps1", bufs=3, space="PSUM") as ps1,
            tc.tile_pool(name="ps2", bufs=3, space="PSUM") as ps2,
            tc.tile_pool(name="psagg", bufs=2, space="PSUM") as psagg,
        ):
            # ---- load constants to SBUF
            def cload(ap_src, shape, tag, dt=F32):
                t = cpool.tile(list(shape), dt, tag=tag)
                nc.sync.dma_start(out=t[:], in_=ap_src)
                return t

            iota_sb = cload(iota_in[:], [P, P], "c_iota")
            ident_sb = cload(ident_in[:], [P, P], "c_ident")
            srct_sb = cload(srctab[:], [P, nch], "c_srct", I32)
            dstt_sb = cload(dsttab[:], [P, nch], "c_dstt")
            fcwt_sb = cload(fcwt[:], [F, C], "c_fcwt")
            fcb_sb = cload(fcbr[:], [1, C], "c_fcb")
            w_sb = {}
            b_sb = {}
            for l in range(L):
                for nm, t in (("q", wqt), ("k", wkt), ("v", wvt), ("s", wst)):
                    w_sb[(nm, l)] = cload(t[l], [F, F], f"c_w{nm}{l}")
                for nm, t in (("q", bqr), ("k", bkr), ("v", bvr), ("s", bsr)):
                    b_sb[(nm, l)] = cload(t[l], [1, F], f"c_b{nm}{l}")
            ones_r = cpool.tile([1, P], F32)
            nc.vector.memset(ones_r[:], 1.0)
            ones_c = cpool.tile([P, 1], F32)
            nc.vector.memset(ones_c[:], 1.0)

            hT_a = bigp.tile([P, NOP], F32, tag="hta")
            hT_b = bigp.tile([P, NOP], F32, tag="htb")
            qT = bigp.tile([P, NOP], F32, tag="qt")
            s_sb = bigp.tile([P, NOP], F32, tag="ssb")
            nc.sync.dma_start(out=hT_a[:], in_=xT[:])

            for l in range(L):
                hT_in = hT_a if l == 0 else hT_b
                hT_out = hT_b if l == 0 else hT_a
                # ---- projections per block
                for b in range(NB):
                    cs = slice(b * P, (b + 1) * P)
                    qps = ps1.tile([P, P], F32, tag="t1")
                    nc.tensor.matmul(qps[:], lhsT=w_sb[("q", l)][:], rhs=hT_in[:, cs], start=True, stop=False)
                    nc.tensor.matmul(qps[:], lhsT=b_sb[("q", l)][:], rhs=ones_r[:], start=False, stop=True)
                    nc.scalar.activation(qT[:, cs], qps[:], mybir.ActivationFunctionType.Copy)

                    sps = ps2.tile([P, P], F32, tag="t2")
                    nc.tensor.matmul(sps[:], lhsT=hT_in[:, cs], rhs=w_sb[("s", l)][:], start=True, stop=False)
                    nc.tensor.matmul(sps[:], lhsT=ones_r[:], rhs=b_sb[("s", l)][:], start=False, stop=True)
                    nc.scalar.activation(s_sb[:, cs], sps[:], mybir.ActivationFunctionType.Copy)

                    for nm, lo in (("k", 0), ("v", F)):
                        kps = ps2.tile([P, P], F32, tag="t2")
                        nc.tensor.matmul(kps[:], lhsT=hT_in[:, cs], rhs=w_sb[(nm, l)][:], start=True, stop=False)
                        nc.tensor.matmul(kps[:], lhsT=ones_r[:], rhs=b_sb[(nm, l)][:], start=False, stop=True)
                        ksb = work.tile([P, P], F32, tag="kvout")
                        nc.vector.tensor_copy(out=ksb[:], in_=kps[:])
                        nc.sync.dma_start(out=kv_own[cs, lo : lo + F], in_=ksb[:])
                    nc.sync.dma_start(out=kv_own[cs, 2 * F : 2 * F + 1], in_=ones_c[:])

                # ---- halo exchange
                nc.gpsimd.collective_compute(
                    "AllGather",
                    mybir.AluOpType.bypass,
                    replica_groups=groups,
                    ins=[kv_own[:]],
                    outs=[kv_all[:]],
                )

                # ---- edge phase
                for b in range(NB):
                    cs = slice(b * P, (b + 1) * P)
                    agg = psagg.tile([P, F + 1], F32, tag="agg")
                    for cc in range(cmax):
                        j = b * cmax + cc
                        kvg = kvp.tile([P, 2 * F + 1], F32, tag="kvg")
                        nc.gpsimd.indirect_dma_start(
                            out=kvg[:],
                            out_offset=None,
                            in_=kv_all[:],
                            in_offset=bass.IndirectOffsetOnAxis(ap=srct_sb[:, j : j + 1], axis=0),
                        )
                        ktp = ps1.tile([P, P], F32, tag="t1")
                        nc.tensor.transpose(ktp[:], kvg[:, 0:F], ident_sb[:])
                        kts = work.tile([P, P], F32, tag="kts")
                        nc.scalar.activation(kts[:], ktp[:], mybir.ActivationFunctionType.Copy)
                        scps = ps2.tile([P, P], F32, tag="t2")
                        nc.tensor.matmul(scps[:], lhsT=kts[:], rhs=qT[:, cs], start=True, stop=True)
                        expS = work.tile([P, P], F32, tag="expS")
                        nc.scalar.activation(expS[:], scps[:], mybir.ActivationFunctionType.Exp, scale=float(SCALE))
                        mask = work.tile([P, P], F32, tag="mask")
                        nc.vector.tensor_tensor(
                            out=mask[:],
                            in0=dstt_sb[:, j : j + 1].to_broadcast([P, P]),
                            in1=iota_sb[:],
                            op=mybir.AluOpType.is_equal,
                        )
                        mw = work.tile([P, P], F32, tag="mw")
                        nc.vector.tensor_tensor(out=mw[:], in0=expS[:], in1=mask[:], op=mybir.AluOpType.mult)
                        nc.tensor.matmul(agg[:, 0 : F + 1], lhsT=mw[:], rhs=kvg[:, F : 2 * F + 1], start=(cc == 0), stop=(cc == cmax - 1))
                    # ---- finalize block
                    dn = work.tile([P, 1], F32, tag="dn")
                    nc.vector.tensor_scalar(dn[:], agg[:, F : F + 1], 1e-30, None, op0=mybir.AluOpType.max)
                    rc = work.tile([P, 1], F32, tag="rc")
                    nc.vector.reciprocal(rc[:], dn[:])
                    hn = work.tile([P, P], F32, tag="hn")
                    nc.scalar.activation(hn[:], agg[:, 0:F], mybir.ActivationFunctionType.Copy, scale=rc[:])
                    hn2 = work.tile([P, P], F32, tag="hn2")
                    nc.vector.tensor_tensor(out=hn2[:], in0=hn[:], in1=s_sb[:, cs], op=mybir.AluOpType.add)
                    hrelu = work.tile([P, P], F32, tag="hrelu")
                    nc.scalar.activation(hrelu[:], hn2[:], mybir.ActivationFunctionType.Relu)
                    htp = ps1.tile([P, P], F32, tag="t1")
                    nc.tensor.transpose(htp[:], hrelu[:], ident_sb[:])
                    nc.vector.tensor_copy(out=hT_out[:, cs], in_=htp[:])

            # ---- FC + log_softmax
            for b in range(NB):
                cs = slice(b * P, (b + 1) * P)
                lg = ps2.tile([P, C], F32, tag="t2")
                nc.tensor.matmul(lg[:], lhsT=hT_a[:, cs], rhs=fcwt_sb[:], start=True, stop=False)
                nc.tensor.matmul(lg[:], lhsT=ones_r[:], rhs=fcb_sb[:], start=False, stop=True)
                expl = work.tile([P, C], F32, tag="expl")
                sume = work.tile([P, 1], F32, tag="sume")
                nc.scalar.activation(expl[:], lg[:], mybir.ActivationFunctionType.Exp, accum_out=sume[:])
                lse = work.tile([P, 1], F32, tag="lse")
                nc.scalar.activation(lse[:], sume[:], mybir.ActivationFunctionType.Ln)
                ot = work.tile([P, C], F32, tag="ot")
                nc.vector.tensor_scalar(ot[:], lg[:], lse[:], None, op0=mybir.AluOpType.subtract)
                nc.sync.dma_start(out=out[cs, :], in_=ot[:])

    nc.compile()
    return nc


def kernel(x, edge_index, Wq, bq, Wk, bk, Wv, bv, Ws, bs, fc_W, fc_b, _want_trace=False):
    x = np.asarray(x, dtype=np.float32)
    cmax, srctabs, dsttabs = _host_prep(edge_index)

    if cmax not in _cache:
        _cache[cmax] = _build(cmax)
    nc = _cache[cmax]

    iota = np.tile(np.arange(P, dtype=np.float32)[None, :], (P, 1))
    ident = np.eye(P, dtype=np.float32)
    shared = {
        "wqt": np.ascontiguousarray(np.transpose(np.asarray(Wq, np.float32), (0, 2, 1))),
        "wkt": np.ascontiguousarray(np.transpose(np.asarray(Wk, np.float32), (0, 2, 1))),
        "wvt": np.ascontiguousarray(np.transpose(np.asarray(Wv, np.float32), (0, 2, 1))),
        "wst": np.ascontiguousarray(np.transpose(np.asarray(Ws, np.float32), (0, 2, 1))),
        "bqr": np.asarray(bq, np.float32).reshape(L, 1, F),
        "bkr": np.asarray(bk, np.float32).reshape(L, 1, F),
        "bvr": np.asarray(bv, np.float32).reshape(L, 1, F),
        "bsr": np.asarray(bs, np.float32).reshape(L, 1, F),
        "fcwt": np.ascontiguousarray(np.asarray(fc_W, np.float32).T),
        "fcbr": np.asarray(fc_b, np.float32).reshape(1, C),
        "iota": iota,
        "ident": ident,
    }
    in_maps = []
    for c in range(M):
        xc = np.zeros((NOP, F), dtype=np.float32)
        xc[:NO] = x[c * NO : (c + 1) * NO]
        m = dict(shared)
        m["xT"] = np.ascontiguousarray(xc.T)
        m["srctab"] = srctabs[c]
        m["dsttab"] = dsttabs[c]
        in_maps.append(m)

    import time as _time

    t0 = _time.perf_counter()
    try:
        res = bass_utils.run_bass_kernel_spmd(
            nc, in_maps, core_ids=list(range(M)), trace=_want_trace
        )
    except ModuleNotFoundError:
        res = bass_utils.run_bass_kernel_spmd(
            nc, in_maps, core_ids=list(range(M)), trace=False
        )
    kernel._exec_wall_ns = (_time.perf_counter() - t0) * 1e9
    outp = np.concatenate([np.asarray(res.results[c]["out"])[:NO] for c in range(M)], axis=0)
    kernel._last_result = res
    return outp



# revision 2
# speedup vs baseline: 9.1743x; 9.1743x over previous
"""Distributed TransformerConv GNN (2 layers + FC + log_softmax) on 8 trn2 cores.

Sharding: nodes partitioned by destination across 8 cores (6250 own nodes each,
padded to 6272 = 49x128). Edges sharded by dst, sorted by dst on host. Per layer:
each core computes q/k/v/s projections for its own nodes, AllGathers the k|v
table, then processes its edges in 128-edge chunks: indirect-DMA gather of kv
rows by src, PE-transpose k, PE matmul scores against blockwise q^T, exp on ACT,
one-hot dst mask (iota compare), masked-exp weights, and PE matmul accumulation
of both the weighted-v aggregate and the softmax denominator in PSUM.
No segment-max is needed: scores are O(1) here, so softmax without max
subtraction is mathematically identical and fp32-safe.

Host path: the PJRT executable (jax.jit of the shard_map'd bass custom call)
is built once per cmax and cached, so repeat kernel() calls skip retrace /
recompile. x ships as fp16 (cast to f32 on device); iota/identity are
generated on device; dsttab ships as int8.
"""

import sys

sys.path.insert(0, "/opt/trn_rl_repo")

import numpy as np

from concourse import bacc, bass, mybir, tile
from concourse import bass2jax

N = 50000
E = 600000
F = 128
C = 10
L = 2
M = 8  # cores
NO = N // M  # 6250 own real nodes
P = 128
NB = (NO + P - 1) // P  # 49 blocks
NOP = NB * P  # 6272 padded own nodes
NPAD = M * NOP  # 50176
SCALE = 1.0 / np.sqrt(128.0)

F32 = mybir.dt.float32
F16 = mybir.dt.float16
I32 = mybir.dt.int32
I8 = mybir.dt.int8

_cache = {}


def _host_prep(edge_index):
    """Sort/bucket edges by dst; build per-core [128, NCH] src-index and
    dst-local tables (column j = chunk j, chunk j = block*(CMAX)+c)."""
    src = np.asarray(edge_index[0], dtype=np.int64)
    dst = np.asarray(edge_index[1], dtype=np.int64)
    src_pad = (src // NO) * NOP + (src % NO)  # remap to padded node ids

    core_of = dst // NO
    srctabs, dsttabs = [], []
    percore = []
    cmax = 1
    for c in range(M):
        sel = core_of == c
        d_loc = (dst[sel] - c * NO).astype(np.int64)
        s_glob = src_pad[sel]
        order = np.argsort(d_loc, kind="stable")
        d_loc = d_loc[order]
        s_glob = s_glob[order]
        cnt = np.bincount(d_loc // P, minlength=NB)
        cmax = max(cmax, int(np.max((cnt + P - 1) // P)))
        percore.append((d_loc, s_glob, cnt))
    for c in range(M):
        d_loc, s_glob, cnt = percore[c]
        nch = NB * cmax
        srctab = np.zeros((nch, P), dtype=np.int32)
        dsttab = np.full((nch, P), -1, dtype=np.int8)
        starts = np.concatenate([[0], np.cumsum(cnt)])
        for b in range(NB):
            e0, e1 = starts[b], starts[b + 1]
            n = e1 - e0
            if n == 0:
                continue
            flat_s = np.zeros(cmax * P, dtype=np.int32)
            flat_d = np.full(cmax * P, -1, dtype=np.int8)
            flat_s[:n] = s_glob[e0:e1]
            flat_d[:n] = (d_loc[e0:e1] % P).astype(np.int8)
            srctab[b * cmax : (b + 1) * cmax] = flat_s.reshape(cmax, P)
            dsttab[b * cmax : (b + 1) * cmax] = flat_d.reshape(cmax, P)
        # transpose so column j = chunk j, partition p = edge p of chunk
        srctabs.append(np.ascontiguousarray(srctab.T))
        dsttabs.append(np.ascontiguousarray(dsttab.T))
    return cmax, srctabs, dsttabs


def _build(cmax):
    nch = NB * cmax
    nc = bacc.Bacc("TRN2", target_bir_lowering=False, debug=False, num_devices=M)

    def din(name, shape, dt=F32):
        return nc.dram_tensor(name, list(shape), dt, kind="ExternalInput").ap()

    xT = din("xT", [P, NOP], F16)
    srctab = din("srctab", [P, nch], I32)
    dsttab = din("dsttab", [P, nch], I8)
    wqt = din("wqt", [L, F, F])
    wkt = din("wkt", [L, F, F])
    wvt = din("wvt", [L, F, F])
    wst = din("wst", [L, F, F])
    bqr = din("bqr", [L, 1, F])
    bkr = din("bkr", [L, 1, F])
    bvr = din("bvr", [L, 1, F])
    bsr = din("bsr", [L, 1, F])
    fcwt = din("fcwt", [F, C])
    fcbr = din("fcbr", [1, C])
    out = nc.dram_tensor("out", [NOP, C], F32, kind="ExternalOutput").ap()

    kv_own = nc.dram_tensor("kv_own", [NOP, 2 * F + 1], F32)
    kv_all = nc.dram_tensor("kv_all", [NPAD, 2 * F + 1], F32, addr_space="Shared")

    groups = [list(range(M))]

    with tile.TileContext(nc) as tc:
        with (
            tc.tile_pool(name="const", bufs=1) as cpool,
            tc.tile_pool(name="big", bufs=1) as bigp,
            tc.tile_pool(name="work", bufs=4) as work,
            tc.tile_pool(name="kvpool", bufs=6) as kvp,
            tc.tile_pool(name="ps1", bufs=3, space="PSUM") as ps1,
            tc.tile_pool(name="ps2", bufs=3, space="PSUM") as ps2,
            tc.tile_pool(name="psagg", bufs=2, space="PSUM") as psagg,
        ):
            # ---- load constants to SBUF
            def cload(ap_src, shape, tag, dt=F32):
                t = cpool.tile(list(shape), dt, tag=tag)
                nc.sync.dma_start(out=t[:], in_=ap_src)
                return t

            srct_sb = cload(srctab[:], [P, nch], "c_srct", I32)
            dstt_i8 = cload(dsttab[:], [P, nch], "c_dstt8", I8)
            dstt_sb = cpool.tile([P, nch], F32, tag="c_dstt")
            nc.vector.tensor_copy(out=dstt_sb[:], in_=dstt_i8[:])
            fcwt_sb = cload(fcwt[:], [F, C], "c_fcwt")
            fcb_sb = cload(fcbr[:], [1, C], "c_fcb")
            # iota row (0..P-1 along free dim) and identity, generated on-chip
            iota_sb = cpool.tile([P, P], F32, tag="c_iota")
            nc.gpsimd.iota(
                iota_sb[:], pattern=[[1, P]], base=0, channel_multiplier=0,
                allow_small_or_imprecise_dtypes=True,
            )
            pidx_sb = cpool.tile([P, 1], F32, tag="c_pidx")
            nc.gpsimd.iota(
                pidx_sb[:], pattern=[[1, 1]], base=0, channel_multiplier=1,
                allow_small_or_imprecise_dtypes=True,
            )
            ident_sb = cpool.tile([P, P], F32, tag="c_ident")
            nc.vector.tensor_tensor(
                out=ident_sb[:],
                in0=iota_sb[:],
                in1=pidx_sb[:].to_broadcast([P, P]),
                op=mybir.AluOpType.is_equal,
            )
            w_sb = {}
            b_sb = {}
            for l in range(L):
                for nm, t in (("q", wqt), ("k", wkt), ("v", wvt), ("s", wst)):
                    w_sb[(nm, l)] = cload(t[l], [F, F], f"c_w{nm}{l}")
                for nm, t in (("q", bqr), ("k", bkr), ("v", bvr), ("s", bsr)):
                    b_sb[(nm, l)] = cload(t[l], [1, F], f"c_b{nm}{l}")
            ones_r = cpool.tile([1, P], F32)
            nc.vector.memset(ones_r[:], 1.0)
            ones_c = cpool.tile([P, 1], F32)
            nc.vector.memset(ones_c[:], 1.0)

            hT_a = bigp.tile([P, NOP], F32, tag="hta")
            hT_b = bigp.tile([P, NOP], F32, tag="htb")
            qT = bigp.tile([P, NOP], F32, tag="qt")
            s_sb = bigp.tile([P, NOP], F32, tag="ssb")
            nc.gpsimd.dma_start(out=hT_a[:], in_=xT[:])  # fp16 -> f32 cast DMA

            for l in range(L):
                hT_in = hT_a if l == 0 else hT_b
                hT_out = hT_b if l == 0 else hT_a
                # ---- projections per block
                for b in range(NB):
                    cs = slice(b * P, (b + 1) * P)
                    qps = ps1.tile([P, P], F32, tag="t1")
                    nc.tensor.matmul(qps[:], lhsT=w_sb[("q", l)][:], rhs=hT_in[:, cs], start=True, stop=False)
                    nc.tensor.matmul(qps[:], lhsT=b_sb[("q", l)][:], rhs=ones_r[:], start=False, stop=True)
                    nc.scalar.activation(qT[:, cs], qps[:], mybir.ActivationFunctionType.Copy)

                    sps = ps2.tile([P, P], F32, tag="t2")
                    nc.tensor.matmul(sps[:], lhsT=hT_in[:, cs], rhs=w_sb[("s", l)][:], start=True, stop=False)
                    nc.tensor.matmul(sps[:], lhsT=ones_r[:], rhs=b_sb[("s", l)][:], start=False, stop=True)
                    nc.scalar.activation(s_sb[:, cs], sps[:], mybir.ActivationFunctionType.Copy)

                    for nm, lo in (("k", 0), ("v", F)):
                        kps = ps2.tile([P, P], F32, tag="t2")
                        nc.tensor.matmul(kps[:], lhsT=hT_in[:, cs], rhs=w_sb[(nm, l)][:], start=True, stop=False)
                        nc.tensor.matmul(kps[:], lhsT=ones_r[:], rhs=b_sb[(nm, l)][:], start=False, stop=True)
                        ksb = work.tile([P, P], F32, tag="kvout")
                        nc.vector.tensor_copy(out=ksb[:], in_=kps[:])
                        nc.sync.dma_start(out=kv_own[cs, lo : lo + F], in_=ksb[:])
                    nc.sync.dma_start(out=kv_own[cs, 2 * F : 2 * F + 1], in_=ones_c[:])

                # ---- halo exchange
                nc.gpsimd.collective_compute(
                    "AllGather",
                    mybir.AluOpType.bypass,
                    replica_groups=groups,
                    ins=[kv_own[:]],
                    outs=[kv_all[:]],
                )

                # ---- edge phase
                for b in range(NB):
                    cs = slice(b * P, (b + 1) * P)
                    agg = psagg.tile([P, F + 1], F32, tag="agg")
                    for cc in range(cmax):
                        j = b * cmax + cc
                        kvg = kvp.tile([P, 2 * F + 1], F32, tag="kvg")
                        nc.gpsimd.indirect_dma_start(
                            out=kvg[:],
                            out_offset=None,
                            in_=kv_all[:],
                            in_offset=bass.IndirectOffsetOnAxis(ap=srct_sb[:, j : j + 1], axis=0),
                        )
                        ktp = ps1.tile([P, P], F32, tag="t1")
                        nc.tensor.transpose(ktp[:], kvg[:, 0:F], ident_sb[:])
                        kts = work.tile([P, P], F32, tag="kts")
                        nc.scalar.activation(kts[:], ktp[:], mybir.ActivationFunctionType.Copy)
                        scps = ps2.tile([P, P], F32, tag="t2")
                        nc.tensor.matmul(scps[:], lhsT=kts[:], rhs=qT[:, cs], start=True, stop=True)
                        expS = work.tile([P, P], F32, tag="expS")
                        nc.scalar.activation(expS[:], scps[:], mybir.ActivationFunctionType.Exp, scale=float(SCALE))
                        mask = work.tile([P, P], F32, tag="mask")
                        nc.vector.tensor_tensor(
                            out=mask[:],
                            in0=dstt_sb[:, j : j + 1].to_broadcast([P, P]),
                            in1=iota_sb[:],
                            op=mybir.AluOpType.is_equal,
                        )
                        mw = work.tile([P, P], F32, tag="mw")
                        nc.vector.tensor_tensor(out=mw[:], in0=expS[:], in1=mask[:], op=mybir.AluOpType.mult)
                        nc.tensor.matmul(agg[:, 0 : F + 1], lhsT=mw[:], rhs=kvg[:, F : 2 * F + 1], start=(cc == 0), stop=(cc == cmax - 1))
                    # ---- finalize block
                    dn = work.tile([P, 1], F32, tag="dn")
                    nc.vector.tensor_scalar(dn[:], agg[:, F : F + 1], 1e-30, None, op0=mybir.AluOpType.max)
                    rc = work.tile([P, 1], F32, tag="rc")
                    nc.vector.reciprocal(rc[:], dn[:])
                    hn = work.tile([P, P], F32, tag="hn")
                    nc.scalar.activation(hn[:], agg[:, 0:F], mybir.ActivationFunctionType.Copy, scale=rc[:])
                    hn2 = work.tile([P, P], F32, tag="hn2")
                    nc.vector.tensor_tensor(out=hn2[:], in0=hn[:], in1=s_sb[:, cs], op=mybir.AluOpType.add)
                    hrelu = work.tile([P, P], F32, tag="hrelu")
                    nc.scalar.activation(hrelu[:], hn2[:], mybir.ActivationFunctionType.Relu)
                    htp = ps1.tile([P, P], F32, tag="t1")
                    nc.tensor.transpose(htp[:], hrelu[:], ident_sb[:])
                    nc.vector.tensor_copy(out=hT_out[:, cs], in_=htp[:])

            # ---- FC + log_softmax
            for b in range(NB):
                cs = slice(b * P, (b + 1) * P)
                lg = ps2.tile([P, C], F32, tag="t2")
                nc.tensor.matmul(lg[:], lhsT=hT_a[:, cs], rhs=fcwt_sb[:], start=True, stop=False)
                nc.tensor.matmul(lg[:], lhsT=ones_r[:], rhs=fcb_sb[:], start=False, stop=True)
                expl = work.tile([P, C], F32, tag="expl")
                sume = work.tile([P, 1], F32, tag="sume")
                nc.scalar.activation(expl[:], lg[:], mybir.ActivationFunctionType.Exp, accum_out=sume[:])
                lse = work.tile([P, 1], F32, tag="lse")
                nc.scalar.activation(lse[:], sume[:], mybir.ActivationFunctionType.Ln)
                ot = work.tile([P, C], F32, tag="ot")
                nc.vector.tensor_scalar(ot[:], lg[:], lse[:], None, op0=mybir.AluOpType.subtract)
                nc.sync.dma_start(out=out[cs, :], in_=ot[:])

    nc.compile()
    return nc


def _make_runner(nc):
    """Build a persistent jitted PJRT runner for the SPMD bass program.

    Replicates bass_utils.run_bass_kernel_spmd's axon path, but the jax.jit
    callable is constructed once and reused, so repeat calls skip
    retrace/recompile (~1.9 s per call saved)."""
    import jax
    from jax.sharding import Mesh, PartitionSpec
    from jax.experimental.shard_map import shard_map

    bass2jax.install_neuronx_cc_hook()

    partition_name = nc.partition_id_tensor.name if nc.partition_id_tensor else None

    in_names, out_names, out_avals, out_shapes = [], [], [], []
    for alloc in nc.m.functions[0].allocations:
        if not isinstance(alloc, mybir.MemoryLocationSet):
            continue
        name = alloc.memorylocations[0].name
        if alloc.kind == "ExternalInput":
            if name != partition_name:
                in_names.append(name)
        elif alloc.kind == "ExternalOutput":
            shape = tuple(alloc.tensor_shape)
            dtype = mybir.dt.np(alloc.dtype)
            out_avals.append(jax.core.ShapedArray(shape, dtype))
            out_shapes.append((shape, dtype))
            out_names.append(name)
    n_params = len(in_names)
    n_outs = len(out_avals)
    in_names_full = list(in_names) + out_names
    if partition_name is not None:
        in_names_full.append(partition_name)

    dbg_zero = None
    if nc.dbg_addr is not None:
        assert not nc.dbg_callbacks
        dbg_zero = np.zeros((1, 2), np.uint32)

    def _body(*args):
        operands = list(args)
        if partition_name is not None:
            operands.append(bass2jax.partition_id_tensor())
        outs = bass2jax._bass_exec_p.bind(
            *operands,
            out_avals=tuple(out_avals),
            in_names=tuple(in_names_full),
            out_names=tuple(out_names),
            lowering_input_output_aliases=(),
            sim_require_finite=True,
            sim_require_nnan=True,
            nc=nc,
        )
        return tuple(outs)

    devices = jax.devices()[:M]
    assert len(devices) == M, f"need {M} devices, have {len(jax.devices())}"
    mesh = Mesh(np.asarray(devices), ("core",))
    in_specs = (PartitionSpec("core"),) * (n_params + n_outs)
    out_specs = (PartitionSpec("core"),) * n_outs
    donate = tuple(range(n_params, n_params + n_outs))
    sharded = jax.jit(
        shard_map(_body, mesh=mesh, in_specs=in_specs, out_specs=out_specs, check_rep=False),
        donate_argnums=donate,
        keep_unused=True,
    )

    def run(in_maps):
        if dbg_zero is not None:
            in_maps = [{**m, nc.dbg_addr.name: dbg_zero} for m in in_maps]
        concat_in = [
            np.concatenate([np.asarray(in_maps[c][name]) for c in range(M)], axis=0)
            for name in in_names
        ]
        zeros = [np.zeros((M * s[0], *s[1:]), dt) for s, dt in out_shapes]
        out_arrs = sharded(*concat_in, *zeros)
        # single blocking pull of the global output
        return {name: np.asarray(out_arrs[i]) for i, name in enumerate(out_names)}

    return run


class _ResultShim:
    exec_time_ns = None
    results = None


def kernel(x, edge_index, Wq, bq, Wk, bk, Wv, bv, Ws, bs, fc_W, fc_b, _want_trace=False):
    x = np.asarray(x, dtype=np.float32)
    cmax, srctabs, dsttabs = _host_prep(edge_index)

    if cmax not in _cache:
        nc = _build(cmax)
        _cache[cmax] = (nc, _make_runner(nc))
    nc, runner = _cache[cmax]

    shared = {
        "wqt": np.ascontiguousarray(np.transpose(np.asarray(Wq, np.float32), (0, 2, 1))),
        "wkt": np.ascontiguousarray(np.transpose(np.asarray(Wk, np.float32), (0, 2, 1))),
        "wvt": np.ascontiguousarray(np.transpose(np.asarray(Wv, np.float32), (0, 2, 1))),
        "wst": np.ascontiguousarray(np.transpose(np.asarray(Ws, np.float32), (0, 2, 1))),
        "bqr": np.asarray(bq, np.float32).reshape(L, 1, F),
        "bkr": np.asarray(bk, np.float32).reshape(L, 1, F),
        "bvr": np.asarray(bv, np.float32).reshape(L, 1, F),
        "bsr": np.asarray(bs, np.float32).reshape(L, 1, F),
        "fcwt": np.ascontiguousarray(np.asarray(fc_W, np.float32).T),
        "fcbr": np.asarray(fc_b, np.float32).reshape(1, C),
    }
    in_maps = []
    for c in range(M):
        xc = np.zeros((NOP, F), dtype=np.float32)
        xc[:NO] = x[c * NO : (c + 1) * NO]
        m = dict(shared)
        m["xT"] = np.ascontiguousarray(xc.T).astype(np.float16)
        m["srctab"] = srctabs[c]
        m["dsttab"] = dsttabs[c]
        in_maps.append(m)

    import time as _time

    t0 = _time.perf_counter()
    outs = runner(in_maps)
    kernel._exec_wall_ns = (_time.perf_counter() - t0) * 1e9
    glob = outs["out"].reshape(M, NOP, C)
    outp = np.ascontiguousarray(glob[:, :NO, :]).reshape(N, C)
    res = _ResultShim()
    res.results = [{"out": glob[c]} for c in range(M)]
    kernel._last_result = res
    return outp


# revision 12
# speedup vs baseline: 11.2421x; 1.2254x over previous
"""Distributed TransformerConv GNN (2 layers + FC + log_softmax) on 8 trn2 cores.

Sharding: nodes partitioned by destination across 8 cores (6250 own nodes each,
padded to 6272 = 49x128). Edges sharded by dst, sorted by dst on host. Per layer:
each core computes q/k/v/s projections for its own nodes, AllGathers the k|v
table, then processes its edges in 128-edge chunks: indirect-DMA gather of kv
rows by src, PE-transpose k, PE matmul scores against blockwise q^T, exp on ACT,
one-hot dst mask (iota compare), masked-exp weights, and PE matmul accumulation
of both the weighted-v aggregate and the softmax denominator in PSUM.
No segment-max is needed: scores are O(1) here, so softmax without max
subtraction is mathematically identical and fp32-safe.

Host path: the PJRT executable (jax.jit of the shard_map'd bass custom call)
is built once per cmax and cached, so repeat kernel() calls skip retrace /
recompile. x ships as fp16 (cast to f32 on device); iota/identity are
generated on device; dsttab ships as int8.
"""

import sys

sys.path.insert(0, "/opt/trn_rl_repo")

import numpy as np

from concourse import bacc, bass, mybir, tile
from concourse import bass2jax

N = 50000
E = 600000
F = 128
C = 10
L = 2
M = 8  # cores
NO = N // M  # 6250 own real nodes
P = 128
NB = (NO + P - 1) // P  # 49 blocks
NOP = NB * P  # 6272 padded own nodes
NPAD = M * NOP  # 50176
SCALE = 1.0 / np.sqrt(128.0)

F32 = mybir.dt.float32
F16 = mybir.dt.float16
I32 = mybir.dt.int32
I8 = mybir.dt.int8
U16 = mybir.dt.uint16

_cache = {}


def _host_prep(edge_index):
    """Sort/bucket edges by dst; build per-core [128, NCH] src-index and
    dst-local tables (column j = chunk j, chunk j = block*(CMAX)+c)."""
    src = np.asarray(edge_index[0], dtype=np.int64)
    dst = np.asarray(edge_index[1], dtype=np.int64)
    src_pad = (src // NO) * NOP + (src % NO)  # remap to padded node ids

    core_of = dst // NO
    srctabs, dsttabs = [], []
    percore = []
    cmax = 1
    for c in range(M):
        sel = core_of == c
        d_loc = (dst[sel] - c * NO).astype(np.int64)
        s_glob = src_pad[sel]
        order = np.argsort(d_loc, kind="stable")
        d_loc = d_loc[order]
        s_glob = s_glob[order]
        cnt = np.bincount(d_loc // P, minlength=NB)
        cmax = max(cmax, int(np.max((cnt + P - 1) // P)))
        percore.append((d_loc, s_glob, cnt))
    for c in range(M):
        d_loc, s_glob, cnt = percore[c]
        nch = NB * cmax
        srctab = np.zeros((nch, P), dtype=np.uint16)
        dsttab = np.full((nch, P), -1, dtype=np.int8)
        starts = np.concatenate([[0], np.cumsum(cnt)])
        for b in range(NB):
            e0, e1 = starts[b], starts[b + 1]
            n = e1 - e0
            if n == 0:
                continue
            flat_s = np.zeros(cmax * P, dtype=np.uint16)
            flat_d = np.full(cmax * P, -1, dtype=np.int8)
            flat_s[:n] = s_glob[e0:e1]
            flat_d[:n] = (d_loc[e0:e1] % P).astype(np.int8)
            srctab[b * cmax : (b + 1) * cmax] = flat_s.reshape(cmax, P)
            dsttab[b * cmax : (b + 1) * cmax] = flat_d.reshape(cmax, P)
        # transpose so column j = chunk j, partition p = edge p of chunk
        srctabs.append(np.ascontiguousarray(srctab.T))
        dsttabs.append(np.ascontiguousarray(dsttab.T))
    return cmax, srctabs, dsttabs


def _build(cmax):
    nch = NB * cmax
    nc = bacc.Bacc("TRN2", target_bir_lowering=False, debug=False, num_devices=M)

    def din(name, shape, dt=F32):
        return nc.dram_tensor(name, list(shape), dt, kind="ExternalInput").ap()

    xT = din("xT", [P, NOP], F16)
    srctab = din("srctab", [P, nch], U16)
    dsttab = din("dsttab", [P, nch], I8)
    wqt = din("wqt", [L, F, F], F16)
    wkt = din("wkt", [L, F, F], F16)
    wvt = din("wvt", [L, F, F], F16)
    wst = din("wst", [L, F, F], F16)
    bqr = din("bqr", [L, 1, F], F16)
    bkr = din("bkr", [L, 1, F], F16)
    bvr = din("bvr", [L, 1, F], F16)
    bsr = din("bsr", [L, 1, F], F16)
    fcwt = din("fcwt", [F, C], F16)
    fcbr = din("fcbr", [1, C], F16)
    out = nc.dram_tensor("out", [NOP, C], F32, kind="ExternalOutput").ap()

    kv_own = nc.dram_tensor("kv_own", [NOP, 2 * F + 1], F32)
    kv_all = nc.dram_tensor("kv_all", [NPAD, 2 * F + 1], F32, addr_space="Shared")

    groups = [list(range(M))]

    with tile.TileContext(nc) as tc:
        with (
            tc.tile_pool(name="const", bufs=1) as cpool,
            tc.tile_pool(name="big", bufs=1) as bigp,
            tc.tile_pool(name="work", bufs=4) as work,
            tc.tile_pool(name="kvpool", bufs=6) as kvp,
            tc.tile_pool(name="ps1", bufs=3, space="PSUM") as ps1,
            tc.tile_pool(name="ps2", bufs=3, space="PSUM") as ps2,
            tc.tile_pool(name="psagg", bufs=2, space="PSUM") as psagg,
        ):
            # ---- load constants to SBUF
            def cload(ap_src, shape, tag, dt=F32):
                t = cpool.tile(list(shape), dt, tag=tag)
                nc.sync.dma_start(out=t[:], in_=ap_src)
                return t

            def cload_cast(ap_src, shape, tag, dt=F32):
                # SWDGE cast-DMA: narrow dtype in DRAM -> f32/i32 in SBUF
                t = cpool.tile(list(shape), dt, tag=tag)
                nc.gpsimd.dma_start(out=t[:], in_=ap_src)
                return t

            srct_sb = cload_cast(srctab[:], [P, nch], "c_srct", I32)
            dstt_sb = cload_cast(dsttab[:], [P, nch], "c_dstt")
            fcwt_sb = cload_cast(fcwt[:], [F, C], "c_fcwt")
            fcb_sb = cload_cast(fcbr[:], [1, C], "c_fcb")
            # iota row (0..P-1 along free dim) and identity, generated on-chip
            iota_sb = cpool.tile([P, P], F32, tag="c_iota")
            nc.gpsimd.iota(
                iota_sb[:], pattern=[[1, P]], base=0, channel_multiplier=0,
                allow_small_or_imprecise_dtypes=True,
            )
            pidx_sb = cpool.tile([P, 1], F32, tag="c_pidx")
            nc.gpsimd.iota(
                pidx_sb[:], pattern=[[1, 1]], base=0, channel_multiplier=1,
                allow_small_or_imprecise_dtypes=True,
            )
            ident_sb = cpool.tile([P, P], F32, tag="c_ident")
            nc.vector.tensor_tensor(
                out=ident_sb[:],
                in0=iota_sb[:],
                in1=pidx_sb[:].to_broadcast([P, P]),
                op=mybir.AluOpType.is_equal,
            )
            w_sb = {}
            b_sb = {}
            for l in range(L):
                for nm, t in (("q", wqt), ("k", wkt), ("v", wvt), ("s", wst)):
                    w_sb[(nm, l)] = cload_cast(t[l], [F, F], f"c_w{nm}{l}")
                for nm, t in (("q", bqr), ("k", bkr), ("v", bvr), ("s", bsr)):
                    b_sb[(nm, l)] = cload_cast(t[l], [1, F], f"c_b{nm}{l}")
            ones_r = cpool.tile([1, P], F32)
            nc.vector.memset(ones_r[:], 1.0)
            ones_c = cpool.tile([P, 1], F32)
            nc.vector.memset(ones_c[:], 1.0)

            hT_a = bigp.tile([P, NOP], F32, tag="hta")
            hT_b = bigp.tile([P, NOP], F32, tag="htb")
            qT = bigp.tile([P, NOP], F32, tag="qt")
            s_sb = bigp.tile([P, NOP], F32, tag="ssb")
            nc.gpsimd.dma_start(out=hT_a[:], in_=xT[:])  # fp16 -> f32 cast DMA

            for l in range(L):
                hT_in = hT_a if l == 0 else hT_b
                hT_out = hT_b if l == 0 else hT_a
                # ---- projections per block
                for b in range(NB):
                    cs = slice(b * P, (b + 1) * P)
                    qps = ps1.tile([P, P], F32, tag="t1")
                    nc.tensor.matmul(qps[:], lhsT=w_sb[("q", l)][:], rhs=hT_in[:, cs], start=True, stop=False)
                    nc.tensor.matmul(qps[:], lhsT=b_sb[("q", l)][:], rhs=ones_r[:], start=False, stop=True)
                    nc.scalar.activation(qT[:, cs], qps[:], mybir.ActivationFunctionType.Copy)

                    sps = ps2.tile([P, P], F32, tag="t2")
                    nc.tensor.matmul(sps[:], lhsT=hT_in[:, cs], rhs=w_sb[("s", l)][:], start=True, stop=False)
                    nc.tensor.matmul(sps[:], lhsT=ones_r[:], rhs=b_sb[("s", l)][:], start=False, stop=True)
                    nc.scalar.activation(s_sb[:, cs], sps[:], mybir.ActivationFunctionType.Copy)

                    for nm, lo in (("k", 0), ("v", F)):
                        kps = ps2.tile([P, P], F32, tag="t2")
                        nc.tensor.matmul(kps[:], lhsT=hT_in[:, cs], rhs=w_sb[(nm, l)][:], start=True, stop=False)
                        nc.tensor.matmul(kps[:], lhsT=ones_r[:], rhs=b_sb[(nm, l)][:], start=False, stop=True)
                        ksb = work.tile([P, P], F32, tag="kvout")
                        nc.vector.tensor_copy(out=ksb[:], in_=kps[:])
                        nc.sync.dma_start(out=kv_own[cs, lo : lo + F], in_=ksb[:])
                    nc.sync.dma_start(out=kv_own[cs, 2 * F : 2 * F + 1], in_=ones_c[:])

                # ---- halo exchange
                nc.gpsimd.collective_compute(
                    "AllGather",
                    mybir.AluOpType.bypass,
                    replica_groups=groups,
                    ins=[kv_own[:]],
                    outs=[kv_all[:]],
                )

                # ---- edge phase
                for b in range(NB):
                    cs = slice(b * P, (b + 1) * P)
                    agg = psagg.tile([P, F + 1], F32, tag="agg")
                    for cc in range(cmax):
                        j = b * cmax + cc
                        kvg = kvp.tile([P, 2 * F + 1], F32, tag="kvg")
                        nc.gpsimd.indirect_dma_start(
                            out=kvg[:],
                            out_offset=None,
                            in_=kv_all[:],
                            in_offset=bass.IndirectOffsetOnAxis(ap=srct_sb[:, j : j + 1], axis=0),
                        )
                        ktp = ps1.tile([P, P], F32, tag="t1")
                        nc.tensor.transpose(ktp[:], kvg[:, 0:F], ident_sb[:])
                        kts = work.tile([P, P], F32, tag="kts")
                        nc.scalar.activation(kts[:], ktp[:], mybir.ActivationFunctionType.Copy)
                        scps = ps2.tile([P, P], F32, tag="t2")
                        nc.tensor.matmul(scps[:], lhsT=kts[:], rhs=qT[:, cs], start=True, stop=True)
                        expS = work.tile([P, P], F32, tag="expS")
                        nc.scalar.activation(expS[:], scps[:], mybir.ActivationFunctionType.Exp, scale=float(SCALE))
                        mask = work.tile([P, P], F32, tag="mask")
                        nc.vector.tensor_tensor(
                            out=mask[:],
                            in0=dstt_sb[:, j : j + 1].to_broadcast([P, P]),
                            in1=iota_sb[:],
                            op=mybir.AluOpType.is_equal,
                        )
                        mw = work.tile([P, P], F32, tag="mw")
                        nc.vector.tensor_tensor(out=mw[:], in0=expS[:], in1=mask[:], op=mybir.AluOpType.mult)
                        nc.tensor.matmul(agg[:, 0 : F + 1], lhsT=mw[:], rhs=kvg[:, F : 2 * F + 1], start=(cc == 0), stop=(cc == cmax - 1))
                    # ---- finalize block
                    dn = work.tile([P, 1], F32, tag="dn")
                    nc.vector.tensor_scalar(dn[:], agg[:, F : F + 1], 1e-30, None, op0=mybir.AluOpType.max)
                    rc = work.tile([P, 1], F32, tag="rc")
                    nc.vector.reciprocal(rc[:], dn[:])
                    hn = work.tile([P, P], F32, tag="hn")
                    nc.scalar.activation(hn[:], agg[:, 0:F], mybir.ActivationFunctionType.Copy, scale=rc[:])
                    hn2 = work.tile([P, P], F32, tag="hn2")
                    nc.vector.tensor_tensor(out=hn2[:], in0=hn[:], in1=s_sb[:, cs], op=mybir.AluOpType.add)
                    hrelu = work.tile([P, P], F32, tag="hrelu")
                    nc.scalar.activation(hrelu[:], hn2[:], mybir.ActivationFunctionType.Relu)
                    htp = ps1.tile([P, P], F32, tag="t1")
                    nc.tensor.transpose(htp[:], hrelu[:], ident_sb[:])
                    nc.vector.tensor_copy(out=hT_out[:, cs], in_=htp[:])

            # ---- FC + log_softmax
            for b in range(NB):
                cs = slice(b * P, (b + 1) * P)
                lg = ps2.tile([P, C], F32, tag="t2")
                nc.tensor.matmul(lg[:], lhsT=hT_a[:, cs], rhs=fcwt_sb[:], start=True, stop=False)
                nc.tensor.matmul(lg[:], lhsT=ones_r[:], rhs=fcb_sb[:], start=False, stop=True)
                expl = work.tile([P, C], F32, tag="expl")
                sume = work.tile([P, 1], F32, tag="sume")
                nc.scalar.activation(expl[:], lg[:], mybir.ActivationFunctionType.Exp, accum_out=sume[:])
                lse = work.tile([P, 1], F32, tag="lse")
                nc.scalar.activation(lse[:], sume[:], mybir.ActivationFunctionType.Ln)
                ot = work.tile([P, C], F32, tag="ot")
                nc.vector.tensor_scalar(ot[:], lg[:], lse[:], None, op0=mybir.AluOpType.subtract)
                nc.sync.dma_start(out=out[cs, :], in_=ot[:])

    nc.compile()
    return nc


def _make_runner(nc):
    """Build a persistent jitted PJRT runner for the SPMD bass program.

    Replicates bass_utils.run_bass_kernel_spmd's axon path, but the jax.jit
    callable is constructed once and reused, so repeat calls skip
    retrace/recompile (~1.9 s per call saved)."""
    import jax
    from jax.sharding import Mesh, PartitionSpec
    from jax.experimental.shard_map import shard_map

    bass2jax.install_neuronx_cc_hook()

    partition_name = nc.partition_id_tensor.name if nc.partition_id_tensor else None

    in_names, out_names, out_avals, out_shapes = [], [], [], []
    for alloc in nc.m.functions[0].allocations:
        if not isinstance(alloc, mybir.MemoryLocationSet):
            continue
        name = alloc.memorylocations[0].name
        if alloc.kind == "ExternalInput":
            if name != partition_name:
                in_names.append(name)
        elif alloc.kind == "ExternalOutput":
            shape = tuple(alloc.tensor_shape)
            dtype = mybir.dt.np(alloc.dtype)
            out_avals.append(jax.core.ShapedArray(shape, dtype))
            out_shapes.append((shape, dtype))
            out_names.append(name)
    n_params = len(in_names)
    n_outs = len(out_avals)
    in_names_full = list(in_names) + out_names
    if partition_name is not None:
        in_names_full.append(partition_name)

    dbg_zero = None
    if nc.dbg_addr is not None:
        assert not nc.dbg_callbacks
        dbg_zero = np.zeros((1, 2), np.uint32)

    def _body(*args):
        operands = list(args)
        if partition_name is not None:
            operands.append(bass2jax.partition_id_tensor())
        outs = bass2jax._bass_exec_p.bind(
            *operands,
            out_avals=tuple(out_avals),
            in_names=tuple(in_names_full),
            out_names=tuple(out_names),
            lowering_input_output_aliases=(),
            sim_require_finite=True,
            sim_require_nnan=True,
            nc=nc,
        )
        return tuple(outs)

    devices = jax.devices()[:M]
    assert len(devices) == M, f"need {M} devices, have {len(jax.devices())}"
    mesh = Mesh(np.asarray(devices), ("core",))
    in_specs = (PartitionSpec("core"),) * (n_params + n_outs)
    out_specs = (PartitionSpec("core"),) * n_outs
    donate = tuple(range(n_params, n_params + n_outs))
    sharded = jax.jit(
        shard_map(_body, mesh=mesh, in_specs=in_specs, out_specs=out_specs, check_rep=False),
        donate_argnums=donate,
        keep_unused=True,
    )

    import concurrent.futures as _cf

    pool = _cf.ThreadPoolExecutor(M)

    def run(in_maps):
        if dbg_zero is not None:
            in_maps = [{**m, nc.dbg_addr.name: dbg_zero} for m in in_maps]
        concat_in = [
            np.concatenate([np.asarray(in_maps[c][name]) for c in range(M)], axis=0)
            for name in in_names
        ]
        zeros = [np.zeros((M * s[0], *s[1:]), dt) for s, dt in out_shapes]
        out_arrs = sharded(*concat_in, *zeros)
        # pull shards in parallel (per-shard fetch has high tunnel latency)
        outs = {}
        for i, name in enumerate(out_names):
            arr = out_arrs[i]
            shards = sorted(arr.addressable_shards, key=lambda s: s.index[0].start or 0)
            parts = list(pool.map(lambda s: np.asarray(s.data), shards))
            outs[name] = np.concatenate(parts, axis=0)
        return outs

    return run


class _ResultShim:
    exec_time_ns = None
    results = None


def kernel(x, edge_index, Wq, bq, Wk, bk, Wv, bv, Ws, bs, fc_W, fc_b, _want_trace=False):
    x = np.asarray(x, dtype=np.float32)
    cmax, srctabs, dsttabs = _host_prep(edge_index)

    if cmax not in _cache:
        nc = _build(cmax)
        _cache[cmax] = (nc, _make_runner(nc))
    nc, runner = _cache[cmax]

    shared = {
        "wqt": np.ascontiguousarray(np.transpose(np.asarray(Wq, np.float32), (0, 2, 1))).astype(np.float16),
        "wkt": np.ascontiguousarray(np.transpose(np.asarray(Wk, np.float32), (0, 2, 1))).astype(np.float16),
        "wvt": np.ascontiguousarray(np.transpose(np.asarray(Wv, np.float32), (0, 2, 1))).astype(np.float16),
        "wst": np.ascontiguousarray(np.transpose(np.asarray(Ws, np.float32), (0, 2, 1))).astype(np.float16),
        "bqr": np.asarray(bq, np.float16).reshape(L, 1, F),
        "bkr": np.asarray(bk, np.float16).reshape(L, 1, F),
        "bvr": np.asarray(bv, np.float16).reshape(L, 1, F),
        "bsr": np.asarray(bs, np.float16).reshape(L, 1, F),
        "fcwt": np.ascontiguousarray(np.asarray(fc_W, np.float32).T).astype(np.float16),
        "fcbr": np.asarray(fc_b, np.float16).reshape(1, C),
    }
    in_maps = []
    for c in range(M):
        xc = np.zeros((NOP, F), dtype=np.float32)
        xc[:NO] = x[c * NO : (c + 1) * NO]
        m = dict(shared)
        m["xT"] = np.ascontiguousarray(xc.T).astype(np.float16)
        m["srctab"] = srctabs[c]
        m["dsttab"] = dsttabs[c]
        in_maps.append(m)

    import time as _time

    t0 = _time.perf_counter()
    outs = runner(in_maps)
    kernel._exec_wall_ns = (_time.perf_counter() - t0) * 1e9
    glob = outs["out"].reshape(M, NOP, C)
    outp = np.ascontiguousarray(glob[:, :NO, :]).reshape(N, C)
    res = _ResultShim()
    res.results = [{"out": glob[c]} for c in range(M)]
    kernel._last_result = res
    return outp
